# revision 14
# baseline (speedup 1.0000x reference)
"""Trainium2 Bass kernel: weighted sliding-window min (STL 'Always' robustness).

out[n, w] = min_k( input[n, 4*w + k] * And_weight[0, k] ),  k in [0, 16)

Strategy (8 NeuronCores, data-parallel over batch N=1024 -> 128 rows/core):
  - Host: cast to bf16, deinterleave each row into 4 phase planes
    P_j[b] = x[4b + j], tile along the block axis with a 4-block halo
    (even tile widths => every plane slot is 4B-aligned => DVE 4x/2x modes),
    and prepend the 16 And_weights (bf16) to each row so the weights ride
    in the same first DMA as plane 0.
  - Device: 16 products p_{o,j} = P_j * c[4o+j] split between VectorE
    (tensor_scalar 4x) and ScalarE (ACTIVATE-with-scale), then a 4-level
    tensor_tensor min tree (bf16 2x_1p) with window shifts folded into
    access-pattern offsets.
  - out[w] = min_o m_o[w+o] where m_o[b] = min_j P_j[b]*c[4o+j]; output is
    written bf16 (exact: a min picks one of the bf16 products) and upcast
    to float32 on the host.

Queue layout: ALL input DMAs ride the Sync HWDGE ring in consumption order
(weights+plane0 first; later tiles as two 2-plane DMAs), leaving the Scalar
ring free so ScalarE spends its sequencer time on ACTIVATE products. The
last tile's L1b is split so only the final half waits for the end of the
ScalarE product chain, and its output is stored in two chunks on the two
rings so the second store's ~2us completion latency overlaps the first's.
"""

import numpy as np

# Problem geometry (hardcoded; harness calls kernel() with these shapes)
N, L = 1024, 8192
K, S = 16, 4
W = (L - K) // S + 1          # 2045 output windows per row
NCORES = 8
ROWS = N // NCORES            # 128 rows per core == SBUF partitions
B = L // S                    # 2048 blocks of 4 per row

import os as _os

NT = int(_os.environ.get("K_NT", "2"))      # number of column tiles
_SPLIT = _os.environ.get("K_SPLIT", "")     # comma list of tile widths (blocks)
if _SPLIT:
    BTS = [int(v) for v in _SPLIT.split(",")]
    assert len(BTS) == NT and sum(BTS) >= W
else:
    _bt = (W + NT - 1) // NT
    _bt += _bt % 2              # even
    BTS = [_bt] * (NT - 1) + [W - _bt * (NT - 1)]
TWS = [bt + 4 for bt in BTS]   # per-tile width in blocks (4-block halo, even)
OFFS = [16 + 4 * sum(TWS[:t]) for t in range(NT)]   # flat offset of tile t
FLAT = 16 + 4 * sum(TWS)

# Which of the 16 products (o, j) run on ScalarE (the rest on VectorE), per
# column tile (hex digits). ScalarE is ~2.9x slower per element but its muls
# run in the shadow of VectorE's min tree.
_ACT_N = [int(c, 16) for c in _os.environ.get("K_ACT", "88")]
CH0 = int(_os.environ.get("K_CH0", "512"))   # first chunk of tile0 plane0
J1Q = _os.environ.get("K_J1Q", "0") == "1"   # plane 1 on the Scalar ring
_ACT_ORDER = [(2, 0), (2, 1), (2, 2), (2, 3), (3, 0), (3, 1), (3, 2), (3, 3),
              (1, 0), (1, 1), (1, 2), (1, 3)]
ACT_MULS = [set(_ACT_ORDER[: _ACT_N[min(t, len(_ACT_N) - 1)]]) for t in range(NT)]

_COMPILED = {}


def _build_bass():
    import concourse.bacc as bacc
    import concourse.mybir as mybir
    from concourse.tile import TileContext

    BF16 = mybir.dt.bfloat16
    F32 = mybir.dt.float32
    MIN = mybir.AluOpType.min

    nc = bacc.Bacc(enable_partition_id=False)
    x = nc.dram_tensor("x", [ROWS, FLAT], BF16, kind="ExternalInput")
    out = nc.dram_tensor("out", [ROWS, W], BF16, kind="ExternalOutput")

    # slot(o, j): plane ordering that keeps every min-tree level a dense
    # step-1 access pattern:
    #   Q = [q0A q1A q0B q1B | q2A q3A q2B q3B]
    #   U = [uA vA uB vB],  R = [r0 r1]
    def slot(o, j):
        return 4 * (o // 2) + 2 * (j // 2) + (o % 2)

    with TileContext(nc) as tc:
        with (
            tc.tile_pool(name="wp", bufs=1) as wp,
            tc.tile_pool(name="xin", bufs=2) as xin,
            tc.tile_pool(name="pa", bufs=2) as pa,
            tc.tile_pool(name="pb", bufs=2) as pb,
            tc.tile_pool(name="qq", bufs=2) as qq,
            tc.tile_pool(name="uu", bufs=2) as uu,
            tc.tile_pool(name="rr", bufs=2) as rr,
            tc.tile_pool(name="oo", bufs=2) as oo,
        ):
            # Dummy first Activation so Bacc hoists the ACT table load to the
            # top of the Scalar queue.
            dummy = wp.tile([ROWS, 1], F32)
            nc.scalar.memzero(dummy[:, :])

            # Input DMAs in consumption order. First DMA: weights + first
            # chunk of plane 0 (16 bf16 weights keep the chunk 4B-aligned at
            # byte offset 32). Plane 1 rides the otherwise-idle Scalar ring
            # so its transfer overlaps the Sync ring's issue serialization.
            TW0 = TWS[0]
            xw0 = xin.tile([ROWS, 16 + CH0], BF16, tag="xw0")
            nc.sync.dma_start(out=xw0[:, :], in_=x[:, 0 : 16 + CH0])
            # tensor_scalar needs an fp32 scalar operand: one tiny cast-copy.
            w_f32 = wp.tile([ROWS, 16], F32)
            nc.vector.tensor_copy(out=w_f32[:, :], in_=xw0[:, 0:16])
            w_sb = w_f32
            x0b = xin.tile([ROWS, TW0 - CH0], BF16, tag="x0b")
            nc.sync.dma_start(out=x0b[:, :], in_=x[:, 16 + CH0 : 16 + TW0])
            x1 = xin.tile([ROWS, TW0], BF16, tag="x1")
            (nc.scalar if J1Q else nc.sync).dma_start(
                out=x1[:, :], in_=x[:, 16 + TW0 : 16 + 2 * TW0]
            )
            x2 = xin.tile([ROWS, TW0], BF16, tag="x2")
            nc.sync.dma_start(out=x2[:, :], in_=x[:, 16 + 2 * TW0 : 16 + 3 * TW0])
            x3 = xin.tile([ROWS, TW0], BF16, tag="x3")
            nc.sync.dma_start(out=x3[:, :], in_=x[:, 16 + 3 * TW0 : 16 + 4 * TW0])
            chunks0 = {
                0: [(0, CH0, xw0[:, 16 : 16 + CH0]), (CH0, TW0, x0b[:, :])],
                1: [(0, TW0, x1[:, :])],
                2: [(0, TW0, x2[:, :])],
                3: [(0, TW0, x3[:, :])],
            }
            xt_rest = []
            for t in range(1, NT):
                TWt = TWS[t]
                x01 = xin.tile([ROWS, 2, TWt], BF16, tag=f"x01_{t}")
                nc.sync.dma_start(
                    out=x01[:, :, :], in_=x[:, OFFS[t] : OFFS[t] + 2 * TWt]
                )
                x23 = xin.tile([ROWS, 2, TWt], BF16, tag=f"x23_{t}")
                nc.sync.dma_start(
                    out=x23[:, :, :],
                    in_=x[:, OFFS[t] + 2 * TWt : OFFS[t] + 4 * TWt],
                )
                xt_rest.append((x01, x23))

            for t in range(NT):
                TW = TWS[t]
                wbase = sum(BTS[:t])
                wcnt = min(BTS[t], W - wbase)

                def plane(j):
                    if t == 0:
                        assert len(chunks0[j]) == 1
                        return chunks0[j][0][2]
                    x01, x23 = xt_rest[t - 1]
                    return x01[:, j, :] if j < 2 else x23[:, j - 2, :]

                A = pa.tile([ROWS, 8, TW], BF16)
                Bb = pb.tile([ROWS, 8, TW], BF16)

                def sc(o, j):
                    return w_sb[:, 4 * o + j : 4 * o + j + 1]

                def emit_mul(o, j, eng, part=None):
                    dst = A if (j % 2 == 0) else Bb
                    s = slot(o, j)
                    if t == 0 and len(chunks0[j]) > 1:
                        parts = chunks0[j] if part is None else [chunks0[j][part]]
                        for lo, hi, ap in parts:
                            if eng == "act":
                                nc.scalar.mul(
                                    out=dst[:, s, lo:hi], in_=ap, mul=sc(o, j)
                                )
                            else:
                                nc.vector.tensor_scalar_mul(
                                    out=dst[:, s, lo:hi], in0=ap, scalar1=sc(o, j)
                                )
                    elif eng == "act":
                        nc.scalar.mul(out=dst[:, s, :], in_=plane(j), mul=sc(o, j))
                    else:
                        nc.vector.tensor_scalar_mul(
                            out=dst[:, s, :], in0=plane(j), scalar1=sc(o, j)
                        )

                Q = qq.tile([ROWS, 8, TW], BF16)

                def l1(sl, sh):
                    nc.vector.tensor_tensor(
                        out=Q[:, sl:sh, :], in0=A[:, sl:sh, :], in1=Bb[:, sl:sh, :],
                        op=MIN,
                    )

                # 16 products p_{o,j} = P_j * c[4o+j]. DVE products j-major
                # (follows DMA arrival); on tile 0 the first L1a half is
                # emitted before the j=3 products so DVE has work while the
                # last plane's DMA completes. ACT products j-major on tile 0
                # (follows DMA arrival), o-major on the last tile (so the
                # L1b sub-splits unblock early); any (1,j) extras first
                # (they feed L1a).
                for j in range(4):
                    if t == 0 and j == 3:
                        l1(0, 2)
                    for o in range(4):
                        if (o, j) not in ACT_MULS[t]:
                            emit_mul(o, j, "dve")
                if t == 0:
                    act_order = [(o, j) for j in range(4) for o in (1, 2, 3)]
                else:
                    act_order = _ACT_ORDER[8:] + _ACT_ORDER[:8]
                for o, j in act_order:
                    if (o, j) in ACT_MULS[t]:
                        emit_mul(o, j, "act")

                # L1: q = min(p_{o, j even}, p_{o, j odd})  (8 planes),
                # split by product origin: slots 0:4 depend only on DVE
                # products, slots 4:8 on the (later) ScalarE chain.
                if t == 0:
                    l1(2, 4)
                else:
                    l1(0, 4)
                if t == NT - 1:
                    # last tile: sub-split so only the final piece waits for
                    # the very end of the ScalarE product chain
                    l1(4, 6)
                    l1(6, 8)
                else:
                    l1(4, 8)

                # L2: fold the o -> o+2 shift (+2 blocks, stays 4B-aligned)
                U = uu.tile([ROWS, 4, TW - 2], BF16)
                nc.vector.tensor_tensor(
                    out=U[:, :, :],
                    in0=Q[:, 0:4, 0 : TW - 2],
                    in1=Q[:, 4:8, 2:TW],
                    op=MIN,
                )

                # L3: A-half vs B-half
                R = rr.tile([ROWS, 2, TW - 2], BF16)
                nc.vector.tensor_tensor(
                    out=R[:, :, :], in0=U[:, 0:2, :], in1=U[:, 2:4, :], op=MIN
                )

                # L4: out[w] = min(r0[w], r1[w+1]).
                ot = oo.tile([ROWS, wcnt], BF16)
                H = wcnt
                if t == NT - 1:
                    # last tile: two chunks on the two HWDGE rings so the
                    # second store's completion latency overlaps the first's.
                    HH = H // 2
                    nc.vector.tensor_tensor(
                        out=ot[:, 0:HH], in0=R[:, 0, 0:HH], in1=R[:, 1, 1 : HH + 1],
                        op=MIN,
                    )
                    nc.sync.dma_start(out=out[:, wbase : wbase + HH], in_=ot[:, 0:HH])
                    nc.vector.tensor_tensor(
                        out=ot[:, HH:H], in0=R[:, 0, HH:H], in1=R[:, 1, HH + 1 : H + 1],
                        op=MIN,
                    )
                    nc.scalar.dma_start(
                        out=out[:, wbase + HH : wbase + H], in_=ot[:, HH:H]
                    )
                else:
                    nc.vector.tensor_tensor(
                        out=ot[:, 0:H], in0=R[:, 0, 0:H], in1=R[:, 1, 1 : H + 1],
                        op=MIN,
                    )
                    nc.sync.dma_start(out=out[:, wbase : wbase + H], in_=ot[:, 0:H])
    nc.finalize()
    return nc


def _host_prep(input_f32, And_weight):
    """Shard + relayout host-side. Returns in_maps for the 8 cores."""
    import ml_dtypes

    xb = np.asarray(input_f32, dtype=np.float32).astype(ml_dtypes.bfloat16)
    # [N, L] -> [N, B, 4] -> [N, 4, B] phase planes
    planes = np.ascontiguousarray(xb.reshape(N, B, S).transpose(0, 2, 1))
    # pad block axis so every tile has its halo
    padB = max(sum(BTS) + 4, B)
    padded = np.zeros((N, S, padB), dtype=ml_dtypes.bfloat16)
    padded[:, :, :B] = planes

    flat = np.zeros((N, FLAT), dtype=ml_dtypes.bfloat16)
    wflat = np.asarray(And_weight, dtype=np.float32).reshape(K).astype(
        ml_dtypes.bfloat16
    )
    flat[:, 0:16] = wflat[None, :]
    for t in range(NT):
        b0 = sum(BTS[:t])
        flat[:, OFFS[t] : OFFS[t] + 4 * TWS[t]] = padded[
            :, :, b0 : b0 + TWS[t]
        ].reshape(N, 4 * TWS[t])

    in_maps = []
    for c in range(NCORES):
        in_maps.append({"x": np.ascontiguousarray(flat[c * ROWS : (c + 1) * ROWS])})
    return in_maps


def _get_nc():
    if "nc" not in _COMPILED:
        _COMPILED["nc"] = _build_bass()
    return _COMPILED["nc"]


def _run(in_maps, trace=False, **kw):
    from concourse.bass_utils import run_bass_kernel_spmd

    nc = _get_nc()
    res = run_bass_kernel_spmd(
        nc, in_maps, core_ids=list(range(NCORES)), trace=trace, **kw
    )
    return res


def kernel(input, And_weight):
    in_maps = _host_prep(input, And_weight)
    res = _run(in_maps, trace=False)
    out = np.concatenate([res.results[c]["out"] for c in range(NCORES)], axis=0)
    return out.astype(np.float32)


# revision 15
# speedup vs baseline: 1.0144x; 1.0144x over previous
"""Trainium2 Bass kernel: weighted sliding-window min (STL 'Always' robustness).

out[n, w] = min_k( input[n, 4*w + k] * And_weight[0, k] ),  k in [0, 16)

Strategy (8 NeuronCores, data-parallel over batch N=1024 -> 128 rows/core):
  - Host: cast to bf16, deinterleave each row into 4 phase planes
    P_j[b] = x[4b + j], tile along the block axis with a 4-block halo
    (even tile widths => every plane slot is 4B-aligned => DVE 4x/2x modes),
    and prepend the 16 And_weights (bf16) to each row so the weights ride
    in the same first DMA as plane 0.
  - Device: 16 products p_{o,j} = P_j * c[4o+j] split between VectorE
    (tensor_scalar 4x) and ScalarE (ACTIVATE-with-scale), then a 4-level
    tensor_tensor min tree (bf16 2x_1p) with window shifts folded into
    access-pattern offsets.
  - out[w] = min_o m_o[w+o] where m_o[b] = min_j P_j[b]*c[4o+j]; output is
    written bf16 (exact: a min picks one of the bf16 products) and upcast
    to float32 on the host.

Queue layout: ALL input DMAs ride the Sync HWDGE ring in consumption order
(weights+plane0 first; later tiles as two 2-plane DMAs), leaving the Scalar
ring free so ScalarE spends its sequencer time on ACTIVATE products. The
last tile's L1b is split so only the final half waits for the end of the
ScalarE product chain, and its output is stored in two chunks on the two
rings so the second store's ~2us completion latency overlaps the first's.
"""

import numpy as np

# Problem geometry (hardcoded; harness calls kernel() with these shapes)
N, L = 1024, 8192
K, S = 16, 4
W = (L - K) // S + 1          # 2045 output windows per row
NCORES = 8
ROWS = N // NCORES            # 128 rows per core == SBUF partitions
B = L // S                    # 2048 blocks of 4 per row

import os as _os

NT = int(_os.environ.get("K_NT", "2"))      # number of column tiles
_SPLIT = _os.environ.get("K_SPLIT", "")     # comma list of tile widths (blocks)
if _SPLIT:
    BTS = [int(v) for v in _SPLIT.split(",")]
    assert len(BTS) == NT and sum(BTS) >= W
else:
    _bt = (W + NT - 1) // NT
    _bt += _bt % 2              # even
    BTS = [_bt] * (NT - 1) + [W - _bt * (NT - 1)]
TWS = [bt + 4 for bt in BTS]   # per-tile width in blocks (4-block halo, even)
OFFS = [16 + 4 * sum(TWS[:t]) for t in range(NT)]   # flat offset of tile t
FLAT = 16 + 4 * sum(TWS)

# Which of the 16 products (o, j) run on ScalarE (the rest on VectorE), per
# column tile (hex digits). ScalarE is ~2.9x slower per element but its muls
# run in the shadow of VectorE's min tree.
_ACT_N = [int(c, 16) for c in _os.environ.get("K_ACT", "88")]
CH0 = int(_os.environ.get("K_CH0", "512"))   # first chunk of tile0 plane0
J1Q = _os.environ.get("K_J1Q", "0") == "1"   # plane 1 on the Scalar ring
HH2 = int(_os.environ.get("K_HH", "0"))      # last tile's final store chunk size
_ACT_ORDER = [(2, 0), (2, 1), (2, 2), (2, 3), (3, 0), (3, 1), (3, 2), (3, 3),
              (1, 0), (1, 1), (1, 2), (1, 3)]
ACT_MULS = [set(_ACT_ORDER[: _ACT_N[min(t, len(_ACT_N) - 1)]]) for t in range(NT)]

_COMPILED = {}


def _build_bass():
    import concourse.bacc as bacc
    import concourse.mybir as mybir
    from concourse.tile import TileContext

    BF16 = mybir.dt.bfloat16
    F32 = mybir.dt.float32
    MIN = mybir.AluOpType.min

    nc = bacc.Bacc(enable_partition_id=False)
    x = nc.dram_tensor("x", [ROWS, FLAT], BF16, kind="ExternalInput")
    out = nc.dram_tensor("out", [ROWS, W], BF16, kind="ExternalOutput")

    # slot(o, j): plane ordering that keeps every min-tree level a dense
    # step-1 access pattern:
    #   Q = [q0A q1A q0B q1B | q2A q3A q2B q3B]
    #   U = [uA vA uB vB],  R = [r0 r1]
    def slot(o, j):
        return 4 * (o // 2) + 2 * (j // 2) + (o % 2)

    with TileContext(nc) as tc:
        with (
            tc.tile_pool(name="wp", bufs=1) as wp,
            tc.tile_pool(name="xin", bufs=2) as xin,
            tc.tile_pool(name="pa", bufs=2) as pa,
            tc.tile_pool(name="pb", bufs=2) as pb,
            tc.tile_pool(name="qq", bufs=2) as qq,
            tc.tile_pool(name="uu", bufs=2) as uu,
            tc.tile_pool(name="rr", bufs=2) as rr,
            tc.tile_pool(name="oo", bufs=2) as oo,
        ):
            # Dummy first Activation so Bacc hoists the ACT table load to the
            # top of the Scalar queue.
            dummy = wp.tile([ROWS, 1], F32)
            nc.scalar.memzero(dummy[:, :])

            # Input DMAs in consumption order. First DMA: weights + first
            # chunk of plane 0 (16 bf16 weights keep the chunk 4B-aligned at
            # byte offset 32). Plane 1 rides the otherwise-idle Scalar ring
            # so its transfer overlaps the Sync ring's issue serialization.
            TW0 = TWS[0]
            xw0 = xin.tile([ROWS, 16 + CH0], BF16, tag="xw0")
            nc.sync.dma_start(out=xw0[:, :], in_=x[:, 0 : 16 + CH0])
            # tensor_scalar needs an fp32 scalar operand: one tiny cast-copy.
            w_f32 = wp.tile([ROWS, 16], F32)
            nc.vector.tensor_copy(out=w_f32[:, :], in_=xw0[:, 0:16])
            w_sb = w_f32
            x0b = xin.tile([ROWS, TW0 - CH0], BF16, tag="x0b")
            nc.sync.dma_start(out=x0b[:, :], in_=x[:, 16 + CH0 : 16 + TW0])
            x1 = xin.tile([ROWS, TW0], BF16, tag="x1")
            (nc.scalar if J1Q else nc.sync).dma_start(
                out=x1[:, :], in_=x[:, 16 + TW0 : 16 + 2 * TW0]
            )
            x2 = xin.tile([ROWS, TW0], BF16, tag="x2")
            nc.sync.dma_start(out=x2[:, :], in_=x[:, 16 + 2 * TW0 : 16 + 3 * TW0])
            x3 = xin.tile([ROWS, TW0], BF16, tag="x3")
            nc.sync.dma_start(out=x3[:, :], in_=x[:, 16 + 3 * TW0 : 16 + 4 * TW0])
            chunks0 = {
                0: [(0, CH0, xw0[:, 16 : 16 + CH0]), (CH0, TW0, x0b[:, :])],
                1: [(0, TW0, x1[:, :])],
                2: [(0, TW0, x2[:, :])],
                3: [(0, TW0, x3[:, :])],
            }
            xt_rest = []
            for t in range(1, NT):
                TWt = TWS[t]
                x01 = xin.tile([ROWS, 2, TWt], BF16, tag=f"x01_{t}")
                nc.sync.dma_start(
                    out=x01[:, :, :], in_=x[:, OFFS[t] : OFFS[t] + 2 * TWt]
                )
                x23 = xin.tile([ROWS, 2, TWt], BF16, tag=f"x23_{t}")
                nc.sync.dma_start(
                    out=x23[:, :, :],
                    in_=x[:, OFFS[t] + 2 * TWt : OFFS[t] + 4 * TWt],
                )
                xt_rest.append((x01, x23))

            for t in range(NT):
                TW = TWS[t]
                wbase = sum(BTS[:t])
                wcnt = min(BTS[t], W - wbase)

                def plane(j):
                    if t == 0:
                        assert len(chunks0[j]) == 1
                        return chunks0[j][0][2]
                    x01, x23 = xt_rest[t - 1]
                    return x01[:, j, :] if j < 2 else x23[:, j - 2, :]

                A = pa.tile([ROWS, 8, TW], BF16)
                Bb = pb.tile([ROWS, 8, TW], BF16)

                def sc(o, j):
                    return w_sb[:, 4 * o + j : 4 * o + j + 1]

                def emit_mul(o, j, eng, part=None):
                    dst = A if (j % 2 == 0) else Bb
                    s = slot(o, j)
                    if t == 0 and len(chunks0[j]) > 1:
                        parts = chunks0[j] if part is None else [chunks0[j][part]]
                        for lo, hi, ap in parts:
                            if eng == "act":
                                nc.scalar.mul(
                                    out=dst[:, s, lo:hi], in_=ap, mul=sc(o, j)
                                )
                            else:
                                nc.vector.tensor_scalar_mul(
                                    out=dst[:, s, lo:hi], in0=ap, scalar1=sc(o, j)
                                )
                    elif eng == "act":
                        nc.scalar.mul(out=dst[:, s, :], in_=plane(j), mul=sc(o, j))
                    else:
                        nc.vector.tensor_scalar_mul(
                            out=dst[:, s, :], in0=plane(j), scalar1=sc(o, j)
                        )

                Q = qq.tile([ROWS, 8, TW], BF16)

                def l1(sl, sh):
                    nc.vector.tensor_tensor(
                        out=Q[:, sl:sh, :], in0=A[:, sl:sh, :], in1=Bb[:, sl:sh, :],
                        op=MIN,
                    )

                # 16 products p_{o,j} = P_j * c[4o+j]. DVE products j-major
                # (follows DMA arrival); on tile 0 the first L1a half is
                # emitted before the j=3 products so DVE has work while the
                # last plane's DMA completes. ACT products j-major on tile 0
                # (follows DMA arrival), o-major on the last tile (so the
                # L1b sub-splits unblock early); any (1,j) extras first
                # (they feed L1a).
                for j in range(4):
                    if t == 0 and j == 3:
                        l1(0, 2)
                    for o in range(4):
                        if (o, j) not in ACT_MULS[t]:
                            emit_mul(o, j, "dve")
                if t == 0:
                    act_order = [(o, j) for j in range(4) for o in (1, 2, 3)]
                else:
                    act_order = _ACT_ORDER[8:] + _ACT_ORDER[:8]
                for o, j in act_order:
                    if (o, j) in ACT_MULS[t]:
                        emit_mul(o, j, "act")

                # L1: q = min(p_{o, j even}, p_{o, j odd})  (8 planes),
                # split by product origin: slots 0:4 depend only on DVE
                # products, slots 4:8 on the (later) ScalarE chain.
                if t == 0:
                    l1(2, 4)
                else:
                    l1(0, 4)
                if t == NT - 1:
                    # last tile: sub-split so only the final piece waits for
                    # the very end of the ScalarE product chain
                    l1(4, 6)
                    l1(6, 8)
                else:
                    l1(4, 8)

                # L2: fold the o -> o+2 shift (+2 blocks, stays 4B-aligned)
                U = uu.tile([ROWS, 4, TW - 2], BF16)
                nc.vector.tensor_tensor(
                    out=U[:, :, :],
                    in0=Q[:, 0:4, 0 : TW - 2],
                    in1=Q[:, 4:8, 2:TW],
                    op=MIN,
                )

                # L3: A-half vs B-half
                R = rr.tile([ROWS, 2, TW - 2], BF16)
                nc.vector.tensor_tensor(
                    out=R[:, :, :], in0=U[:, 0:2, :], in1=U[:, 2:4, :], op=MIN
                )

                # L4: out[w] = min(r0[w], r1[w+1]).
                ot = oo.tile([ROWS, wcnt], BF16)
                H = wcnt
                if t == NT - 1:
                    # last tile: two chunks on the two HWDGE rings so the
                    # second store's completion latency overlaps the first's.
                    HH = (H - HH2) if HH2 else (H // 2)
                    nc.vector.tensor_tensor(
                        out=ot[:, 0:HH], in0=R[:, 0, 0:HH], in1=R[:, 1, 1 : HH + 1],
                        op=MIN,
                    )
                    nc.sync.dma_start(out=out[:, wbase : wbase + HH], in_=ot[:, 0:HH])
                    nc.vector.tensor_tensor(
                        out=ot[:, HH:H], in0=R[:, 0, HH:H], in1=R[:, 1, HH + 1 : H + 1],
                        op=MIN,
                    )
                    nc.scalar.dma_start(
                        out=out[:, wbase + HH : wbase + H], in_=ot[:, HH:H]
                    )
                else:
                    nc.vector.tensor_tensor(
                        out=ot[:, 0:H], in0=R[:, 0, 0:H], in1=R[:, 1, 1 : H + 1],
                        op=MIN,
                    )
                    nc.sync.dma_start(out=out[:, wbase : wbase + H], in_=ot[:, 0:H])
    nc.finalize()
    return nc


def _host_prep(input_f32, And_weight):
    """Shard + relayout host-side. Returns in_maps for the 8 cores."""
    import ml_dtypes

    xb = np.asarray(input_f32, dtype=np.float32).astype(ml_dtypes.bfloat16)
    # [N, L] -> [N, B, 4] -> [N, 4, B] phase planes
    planes = np.ascontiguousarray(xb.reshape(N, B, S).transpose(0, 2, 1))
    # pad block axis so every tile has its halo
    padB = max(sum(BTS) + 4, B)
    padded = np.zeros((N, S, padB), dtype=ml_dtypes.bfloat16)
    padded[:, :, :B] = planes

    flat = np.zeros((N, FLAT), dtype=ml_dtypes.bfloat16)
    wflat = np.asarray(And_weight, dtype=np.float32).reshape(K).astype(
        ml_dtypes.bfloat16
    )
    flat[:, 0:16] = wflat[None, :]
    for t in range(NT):
        b0 = sum(BTS[:t])
        flat[:, OFFS[t] : OFFS[t] + 4 * TWS[t]] = padded[
            :, :, b0 : b0 + TWS[t]
        ].reshape(N, 4 * TWS[t])

    in_maps = []
    for c in range(NCORES):
        in_maps.append({"x": np.ascontiguousarray(flat[c * ROWS : (c + 1) * ROWS])})
    return in_maps


def _get_nc():
    if "nc" not in _COMPILED:
        _COMPILED["nc"] = _build_bass()
    return _COMPILED["nc"]


def _run(in_maps, trace=False, **kw):
    from concourse.bass_utils import run_bass_kernel_spmd

    nc = _get_nc()
    res = run_bass_kernel_spmd(
        nc, in_maps, core_ids=list(range(NCORES)), trace=trace, **kw
    )
    return res


def kernel(input, And_weight):
    in_maps = _host_prep(input, And_weight)
    res = _run(in_maps, trace=False)
    out = np.concatenate([res.results[c]["out"] for c in range(NCORES)], axis=0)
    return out.astype(np.float32)


# revision 17
# speedup vs baseline: 1.0444x; 1.0295x over previous
"""Trainium2 Bass kernel: weighted sliding-window min (STL 'Always' robustness).

out[n, w] = min_k( input[n, 4*w + k] * And_weight[0, k] ),  k in [0, 16)

Strategy (8 NeuronCores, data-parallel over batch N=1024 -> 128 rows/core):
  - Host: cast to bf16, deinterleave each row into 4 phase planes
    P_j[b] = x[4b + j], tile along the block axis with a 4-block halo
    (even tile widths => every plane slot is 4B-aligned => DVE 4x/2x modes),
    and prepend the 16 fp32 And_weights bit-packed into 32 bf16 slots so
    the weights ride in the same first DMA as plane 0 and are read back
    on-device via bitcast APs (no cast op, full fp32 weight precision).
  - Device: 16 products p_{o,j} = P_j * c[4o+j] split between VectorE
    (tensor_scalar 4x) and ScalarE (ACTIVATE-with-scale), then a 4-level
    tensor_tensor min tree (bf16 2x_1p) with window shifts folded into
    access-pattern offsets.
  - out[w] = min_o m_o[w+o] where m_o[b] = min_j P_j[b]*c[4o+j]; output is
    written bf16 (exact: a min picks one of the bf16 products) and upcast
    to float32 on the host.

Queue layout: ALL input DMAs ride the Sync HWDGE ring in consumption order
(weights+plane0 first; later tiles as two 2-plane DMAs), leaving the Scalar
ring free so ScalarE spends its sequencer time on ACTIVATE products. The
last tile's L1b is split so only the final half waits for the end of the
ScalarE product chain, and its output is stored in two chunks on the two
rings so the second store's ~2us completion latency overlaps the first's.
"""

import numpy as np

# Problem geometry (hardcoded; harness calls kernel() with these shapes)
N, L = 1024, 8192
K, S = 16, 4
W = (L - K) // S + 1          # 2045 output windows per row
NCORES = 8
ROWS = N // NCORES            # 128 rows per core == SBUF partitions
B = L // S                    # 2048 blocks of 4 per row

import os as _os

NT = int(_os.environ.get("K_NT", "2"))      # number of column tiles
_SPLIT = _os.environ.get("K_SPLIT", "")     # comma list of tile widths (blocks)
if _SPLIT:
    BTS = [int(v) for v in _SPLIT.split(",")]
    assert len(BTS) == NT and sum(BTS) >= W
else:
    _bt = (W + NT - 1) // NT
    _bt += _bt % 2              # even
    BTS = [_bt] * (NT - 1) + [W - _bt * (NT - 1)]
TWS = [bt + 4 for bt in BTS]   # per-tile width in blocks (4-block halo, even)
OFFS = [32 + 4 * sum(TWS[:t]) for t in range(NT)]   # flat offset of tile t
FLAT = 32 + 4 * sum(TWS)   # 32 bf16 slots = the 16 fp32 weights, bit-packed

# Which of the 16 products (o, j) run on ScalarE (the rest on VectorE), per
# column tile (hex digits). ScalarE is ~2.9x slower per element but its muls
# run in the shadow of VectorE's min tree.
_ACT_N = [int(c, 16) for c in _os.environ.get("K_ACT", "88")]
CH0 = int(_os.environ.get("K_CH0", "512"))   # first chunk of tile0 plane0
J1Q = _os.environ.get("K_J1Q", "0") == "1"   # plane 1 on the Scalar ring
HH2 = int(_os.environ.get("K_HH", "0"))      # last tile's final store chunk size
_ACT_ORDER = [(2, 0), (2, 1), (2, 2), (2, 3), (3, 0), (3, 1), (3, 2), (3, 3),
              (1, 0), (1, 1), (1, 2), (1, 3)]
ACT_MULS = [set(_ACT_ORDER[: _ACT_N[min(t, len(_ACT_N) - 1)]]) for t in range(NT)]

_COMPILED = {}


def _build_bass():
    import concourse.bacc as bacc
    import concourse.mybir as mybir
    from concourse.tile import TileContext

    BF16 = mybir.dt.bfloat16
    F32 = mybir.dt.float32
    MIN = mybir.AluOpType.min

    nc = bacc.Bacc(enable_partition_id=False)
    x = nc.dram_tensor("x", [ROWS, FLAT], BF16, kind="ExternalInput")
    out = nc.dram_tensor("out", [ROWS, W], BF16, kind="ExternalOutput")

    # slot(o, j): plane ordering that keeps every min-tree level a dense
    # step-1 access pattern:
    #   Q = [q0A q1A q0B q1B | q2A q3A q2B q3B]
    #   U = [uA vA uB vB],  R = [r0 r1]
    def slot(o, j):
        return 4 * (o // 2) + 2 * (j // 2) + (o % 2)

    with TileContext(nc) as tc:
        with (
            tc.tile_pool(name="wp", bufs=1) as wp,
            tc.tile_pool(name="xin", bufs=2) as xin,
            tc.tile_pool(name="pa", bufs=2) as pa,
            tc.tile_pool(name="pb", bufs=2) as pb,
            tc.tile_pool(name="qq", bufs=2) as qq,
            tc.tile_pool(name="uu", bufs=2) as uu,
            tc.tile_pool(name="rr", bufs=2) as rr,
            tc.tile_pool(name="oo", bufs=2) as oo,
        ):
            # Dummy first Activation so Bacc hoists the ACT table load to the
            # top of the Scalar queue.
            dummy = wp.tile([ROWS, 1], F32)
            nc.scalar.memzero(dummy[:, :])

            # Input DMAs in consumption order. First DMA: weights + first
            # chunk of plane 0. The 16 fp32 weights ride bit-packed in 32
            # bf16 slots (64 B, keeps the chunk 4B-aligned) and are read
            # back on-device via bitcast APs -- no cast op, full precision.
            TW0 = TWS[0]
            xw0 = xin.tile([ROWS, 32 + CH0], BF16, tag="xw0")
            nc.sync.dma_start(out=xw0[:, :], in_=x[:, 0 : 32 + CH0])
            x0b = xin.tile([ROWS, TW0 - CH0], BF16, tag="x0b")
            nc.sync.dma_start(out=x0b[:, :], in_=x[:, 32 + CH0 : 32 + TW0])
            x1 = xin.tile([ROWS, TW0], BF16, tag="x1")
            (nc.scalar if J1Q else nc.sync).dma_start(
                out=x1[:, :], in_=x[:, 32 + TW0 : 32 + 2 * TW0]
            )
            x2 = xin.tile([ROWS, TW0], BF16, tag="x2")
            nc.sync.dma_start(out=x2[:, :], in_=x[:, 32 + 2 * TW0 : 32 + 3 * TW0])
            x3 = xin.tile([ROWS, TW0], BF16, tag="x3")
            nc.sync.dma_start(out=x3[:, :], in_=x[:, 32 + 3 * TW0 : 32 + 4 * TW0])
            chunks0 = {
                0: [(0, CH0, xw0[:, 32 : 32 + CH0]), (CH0, TW0, x0b[:, :])],
                1: [(0, TW0, x1[:, :])],
                2: [(0, TW0, x2[:, :])],
                3: [(0, TW0, x3[:, :])],
            }
            xt_rest = []
            for t in range(1, NT):
                TWt = TWS[t]
                x01 = xin.tile([ROWS, 2, TWt], BF16, tag=f"x01_{t}")
                nc.sync.dma_start(
                    out=x01[:, :, :], in_=x[:, OFFS[t] : OFFS[t] + 2 * TWt]
                )
                x23 = xin.tile([ROWS, 2, TWt], BF16, tag=f"x23_{t}")
                nc.sync.dma_start(
                    out=x23[:, :, :],
                    in_=x[:, OFFS[t] + 2 * TWt : OFFS[t] + 4 * TWt],
                )
                xt_rest.append((x01, x23))

            for t in range(NT):
                TW = TWS[t]
                wbase = sum(BTS[:t])
                wcnt = min(BTS[t], W - wbase)

                def plane(j):
                    if t == 0:
                        assert len(chunks0[j]) == 1
                        return chunks0[j][0][2]
                    x01, x23 = xt_rest[t - 1]
                    return x01[:, j, :] if j < 2 else x23[:, j - 2, :]

                A = pa.tile([ROWS, 8, TW], BF16)
                Bb = pb.tile([ROWS, 8, TW], BF16)

                def sc(o, j):
                    k = 4 * o + j
                    return xw0[:, 2 * k : 2 * k + 2].bitcast(F32)

                def emit_mul(o, j, eng, part=None):
                    dst = A if (j % 2 == 0) else Bb
                    s = slot(o, j)
                    if t == 0 and len(chunks0[j]) > 1:
                        parts = chunks0[j] if part is None else [chunks0[j][part]]
                        for lo, hi, ap in parts:
                            if eng == "act":
                                nc.scalar.mul(
                                    out=dst[:, s, lo:hi], in_=ap, mul=sc(o, j)
                                )
                            else:
                                nc.vector.tensor_scalar_mul(
                                    out=dst[:, s, lo:hi], in0=ap, scalar1=sc(o, j)
                                )
                    elif eng == "act":
                        nc.scalar.mul(out=dst[:, s, :], in_=plane(j), mul=sc(o, j))
                    else:
                        nc.vector.tensor_scalar_mul(
                            out=dst[:, s, :], in0=plane(j), scalar1=sc(o, j)
                        )

                Q = qq.tile([ROWS, 8, TW], BF16)

                def l1(sl, sh):
                    nc.vector.tensor_tensor(
                        out=Q[:, sl:sh, :], in0=A[:, sl:sh, :], in1=Bb[:, sl:sh, :],
                        op=MIN,
                    )

                # 16 products p_{o,j} = P_j * c[4o+j]. DVE products j-major
                # (follows DMA arrival); on tile 0 the first L1a half is
                # emitted before the j=3 products so DVE has work while the
                # last plane's DMA completes. ACT products j-major on tile 0
                # (follows DMA arrival), o-major on the last tile (so the
                # L1b sub-splits unblock early); any (1,j) extras first
                # (they feed L1a).
                for j in range(4):
                    if t == 0 and j == 3:
                        l1(0, 2)
                    for o in range(4):
                        if (o, j) not in ACT_MULS[t]:
                            emit_mul(o, j, "dve")
                if t == 0:
                    act_order = [(o, j) for j in range(4) for o in (1, 2, 3)]
                else:
                    act_order = _ACT_ORDER[8:] + _ACT_ORDER[:8]
                for o, j in act_order:
                    if (o, j) in ACT_MULS[t]:
                        emit_mul(o, j, "act")

                # L1: q = min(p_{o, j even}, p_{o, j odd})  (8 planes),
                # split by product origin: slots 0:4 depend only on DVE
                # products, slots 4:8 on the (later) ScalarE chain.
                if t == 0:
                    l1(2, 4)
                else:
                    l1(0, 4)
                if t == NT - 1:
                    # last tile: sub-split so only the final piece waits for
                    # the very end of the ScalarE product chain
                    l1(4, 6)
                    l1(6, 8)
                else:
                    l1(4, 8)

                # L2: fold the o -> o+2 shift (+2 blocks, stays 4B-aligned)
                U = uu.tile([ROWS, 4, TW - 2], BF16)
                nc.vector.tensor_tensor(
                    out=U[:, :, :],
                    in0=Q[:, 0:4, 0 : TW - 2],
                    in1=Q[:, 4:8, 2:TW],
                    op=MIN,
                )

                # L3: A-half vs B-half
                R = rr.tile([ROWS, 2, TW - 2], BF16)
                nc.vector.tensor_tensor(
                    out=R[:, :, :], in0=U[:, 0:2, :], in1=U[:, 2:4, :], op=MIN
                )

                # L4: out[w] = min(r0[w], r1[w+1]).
                ot = oo.tile([ROWS, wcnt], BF16)
                H = wcnt
                if t == NT - 1:
                    # last tile: two chunks on the two HWDGE rings so the
                    # second store's completion latency overlaps the first's.
                    HH = (H - HH2) if HH2 else (H // 2)
                    nc.vector.tensor_tensor(
                        out=ot[:, 0:HH], in0=R[:, 0, 0:HH], in1=R[:, 1, 1 : HH + 1],
                        op=MIN,
                    )
                    nc.sync.dma_start(out=out[:, wbase : wbase + HH], in_=ot[:, 0:HH])
                    nc.vector.tensor_tensor(
                        out=ot[:, HH:H], in0=R[:, 0, HH:H], in1=R[:, 1, HH + 1 : H + 1],
                        op=MIN,
                    )
                    nc.scalar.dma_start(
                        out=out[:, wbase + HH : wbase + H], in_=ot[:, HH:H]
                    )
                else:
                    nc.vector.tensor_tensor(
                        out=ot[:, 0:H], in0=R[:, 0, 0:H], in1=R[:, 1, 1 : H + 1],
                        op=MIN,
                    )
                    nc.sync.dma_start(out=out[:, wbase : wbase + H], in_=ot[:, 0:H])
    nc.finalize()
    return nc


def _host_prep(input_f32, And_weight):
    """Shard + relayout host-side. Returns in_maps for the 8 cores."""
    import ml_dtypes

    xb = np.asarray(input_f32, dtype=np.float32).astype(ml_dtypes.bfloat16)
    # [N, L] -> [N, B, 4] -> [N, 4, B] phase planes
    planes = np.ascontiguousarray(xb.reshape(N, B, S).transpose(0, 2, 1))
    # pad block axis so every tile has its halo
    padB = max(sum(BTS) + 4, B)
    padded = np.zeros((N, S, padB), dtype=ml_dtypes.bfloat16)
    padded[:, :, :B] = planes

    flat = np.zeros((N, FLAT), dtype=ml_dtypes.bfloat16)
    wbits = (
        np.asarray(And_weight, dtype=np.float32)
        .reshape(K)
        .view(np.uint16)
        .view(ml_dtypes.bfloat16)
    )
    flat[:, 0:32] = wbits[None, :]
    for t in range(NT):
        b0 = sum(BTS[:t])
        flat[:, OFFS[t] : OFFS[t] + 4 * TWS[t]] = padded[
            :, :, b0 : b0 + TWS[t]
        ].reshape(N, 4 * TWS[t])

    in_maps = []
    for c in range(NCORES):
        in_maps.append({"x": np.ascontiguousarray(flat[c * ROWS : (c + 1) * ROWS])})
    return in_maps


def _get_nc():
    if "nc" not in _COMPILED:
        _COMPILED["nc"] = _build_bass()
    return _COMPILED["nc"]


def _run(in_maps, trace=False, **kw):
    from concourse.bass_utils import run_bass_kernel_spmd

    nc = _get_nc()
    res = run_bass_kernel_spmd(
        nc, in_maps, core_ids=list(range(NCORES)), trace=trace, **kw
    )
    return res


def kernel(input, And_weight):
    in_maps = _host_prep(input, And_weight)
    res = _run(in_maps, trace=False)
    out = np.concatenate([res.results[c]["out"] for c in range(NCORES)], axis=0)
    return out.astype(np.float32)
